# revision 20
# baseline (speedup 1.0000x reference)
"""Haar DWT (512x512, levels=1) on 8 Trainium2 NeuronCores.

Input  x: [8, 64, 512, 512] f32  (plus the four Haar band matrices, which
are fixed/deterministic and therefore folded into the kernel math).
Output: (LL, LH, HL, HH), each [8, 64, 256, 256] f32.

Strategy: pure data parallel over the batch dim (core i handles x[i]).
The Haar /2 is folded into the host-side cast (x*0.5, exact).

Two cooperating pipelines (per core), both fed by host-side column
deinterleaving (a pure permutation folded into the cast/copy pass):

* fat path (44 images, units of 4): even/odd column halves stored
  contiguously, so the horizontal butterfly is `even +- odd` on
  unit-stride fp16 (DVE 2x mode) and the vertical butterfly pairs
  adjacent rows within a partition (also 2x). 6 DVE ops per unit,
  ~9.3us. Rows 0..7 of every 16-row group ship their even-column half
  as fp8-e4m3, upcast to fp16 on ACT before the butterflies: exactly 2
  of 4 butterfly inputs fp8 for half the outputs -> rel_err ~1.33e-2
  (sim+HW confirmed), evenly spread over the bands, under the 2e-2
  gate. Input traffic 32MB -> 29.25MB/core.
* pe path (20 images, units of 2): rows-on-partitions load, PE vertical
  butterfly as a [128,128] +-1 band matmul (two band rows per psum
  partition), ACT evacuates PSUM->fp16, DVE does ONLY the horizontal
  `even +- odd` (2x mode thanks to per-row column deinterleave) --
  1.25us/img of DVE instead of 2.3.

DVE busy ~129us, ACT ~85us, PE ~50us, DMA ~61MB at the measured
~420GB/s/core cap -> ~150us; the mix keeps every engine just under the
DMA roofline.

Hard-won scheduling facts (measured): DVE ops need <=2 free AP dims or
they drop out of 2x mode; GpSimd must stay COMPLETELY idle (any op or
DMA trigger there costs ~+30us -- the Q7 cores back the DMA descriptor
path); fio bufs=3 is load-bearing (bufs=2 serializes, +30us); 4KB DMA
runs beat 8KB; one merged store dma_start per unit on the scalar queue
beats fine-grained or multi-queue stores; each unit's ACT work (upcast,
evac) is emitted one unit ahead of the previous unit's store trigger to
avoid ACT head-of-line blocking.
"""

import numpy as np


def _ensure_concourse():
    try:
        import concourse.bass  # noqa: F401
    except ImportError:
        import sys

        for p in ("/opt/trn_rl_repo", "/root/.axon_site/_ro/trn_rl_repo"):
            if p not in sys.path:
                sys.path.append(p)
        import concourse.bass  # noqa: F401


N_CORES = 8
IMG = 512  # image height == width
BANDS = ("ll", "lh", "hl", "hh")
# band order inside the merged fat-path output tensor
BAND_IDX = {"ll": 0, "lh": 1, "hl": 2, "hh": 3}

N_FAT = 44      # images through the fat path (units of 4)
N_PE = 20       # images through the pe path (units of 2)
R8 = 8          # rows per 16-row group whose even-col half ships as fp8
NF8 = R8 * 256  # fp8 elems per partition (upcast target xt[:, :NF8])
NFF = 8192 - NF8  # fp16 elems per partition loaded directly


def make_w():
    """[128,128] fp16 weights [W_e | W_o] for the PE vertical stage.

    W_e col m<32 sums input row pair (4m, 4m+1) -> EVEN band row 2m of L;
    col 32+m difs the same pair -> even band row of H. W_o handles the odd
    band rows (pairs (4m+2, 4m+3)). Using both per psum tile puts TWO
    consecutive band rows on each psum partition (free halves)."""
    w = np.zeros((128, 128), dtype=np.float16)
    for m in range(32):
        for par, base in ((0, 0), (1, 64)):
            r0 = 4 * m + 2 * par
            w[r0, base + m] = 1.0
            w[r0 + 1, base + m] = 1.0
            w[r0, base + 32 + m] = 1.0
            w[r0 + 1, base + 32 + m] = -1.0
    return w


def build_nc(n_images=64):
    """Build the single-core Bass program (SPMD: same program on all cores)."""
    _ensure_concourse()
    from concourse import bacc, mybir
    from concourse.tile import TileContext

    f16 = mybir.dt.float16
    f32 = mybir.dt.float32
    f8 = mybir.dt.float8e4
    # NOTE: keep enable_partition_id at its default (True). Building with
    # False removes a ~3.7 us preamble TENSOR_LOAD but the axon PJRT execute
    # path requires the trailing partition-id parameter and the NEFF faults
    # with NRT_EXEC_UNIT_UNRECOVERABLE without it.
    nc = bacc.Bacc("TRN2", target_bir_lowering=False, debug=False)

    # fat-path inputs, per 4-image unit partition (c g) of 128:
    #   x8: rows 0..R8-1 even cols, fp8       -> NF8 B contiguous/partition
    #   xf: rows R8..15 even cols ++ all odd cols, fp16 -> 2*NFF B contiguous
    x8 = nc.dram_tensor("x8", [N_FAT, 32, NF8], f8, kind="ExternalInput")
    xf = nc.dram_tensor("xf", [N_FAT, 32, NFF], f16, kind="ExternalInput")
    # pe-path input: row-major, each row stored [even cols | odd cols]
    xp = nc.dram_tensor("xp", [N_PE, IMG, IMG], f16, kind="ExternalInput")
    wm = nc.dram_tensor("wm", [128, 128], f16, kind="ExternalInput")
    o = nc.dram_tensor("o", [4, n_images, IMG // 2, IMG // 2], f16,
                       kind="ExternalOutput")

    CI = 4
    FX = 2048 * CI  # free elems per partition of the assembled input tile

    with TileContext(nc) as tc:
        with (
            tc.tile_pool(name="const", bufs=1) as const_pool,
            tc.tile_pool(name="fio", bufs=3) as fio_pool,
            tc.tile_pool(name="f8io", bufs=3) as f8_pool,
            tc.tile_pool(name="fmid", bufs=3) as fmid_pool,
            tc.tile_pool(name="fws", bufs=3) as fws_pool,
            tc.tile_pool(name="pxin", bufs=4) as px_pool,
            tc.tile_pool(name="pev", bufs=2) as pev_pool,
            tc.tile_pool(name="pout", bufs=2) as pout_pool,
            tc.tile_pool(name="ps", bufs=4, space="PSUM") as ps_pool,
        ):
            wt = const_pool.tile([128, 128], f16, tag="w")
            nc.sync.dma_start(out=wt[:], in_=wm[:])

            def emit_fat_load(i0):
                """Fat-path load + ACT upcast (one unit ahead of its
                compute/store block: ACT head-of-line blocking otherwise)."""
                xt = fio_pool.tile([128, FX], f16, tag="x")
                x8t = f8_pool.tile([128, NF8], f8, tag="x8")
                xv8 = x8[i0 : i0 + CI].rearrange("c g m -> (c g) m")
                nc.sync.dma_start(out=x8t[:], in_=xv8)
                nc.scalar.copy(out=xt[:, :NF8], in_=x8t[:])
                xvf = xf[i0 : i0 + CI].rearrange("c g m -> (c g) m")
                for k in range(NFF // 2048):
                    nc.sync.dma_start(
                        out=xt[:, NF8 + k * 2048 : NF8 + (k + 1) * 2048],
                        in_=xvf[:, k * 2048 : (k + 1) * 2048],
                    )
                return xt

            def emit_fat_main(i0, xt):
                # horizontal butterfly: even half +- odd half (DVE 2x mode)
                xtv = xt[:].rearrange("p (eo m) -> p eo m", eo=2)
                cs = fmid_pool.tile([128, FX // 2], f16, tag="cs")
                cd = fmid_pool.tile([128, FX // 2], f16, tag="cd")
                nc.vector.tensor_add(cs[:], xtv[:, 0], xtv[:, 1])
                nc.vector.tensor_sub(cd[:], xtv[:, 0], xtv[:, 1])

                # vertical butterfly: adjacent row pairs within a partition
                ws = fws_pool.tile([128, FX], f16, tag="ws")
                wv = ws[:].rearrange("p (b j w) -> p b j w", b=4, w=256)
                c4 = cs[:].rearrange("p (j eo w) -> p j eo w", eo=2, w=256)
                d4 = cd[:].rearrange("p (j eo w) -> p j eo w", eo=2, w=256)
                nc.vector.tensor_add(wv[:, 0], c4[:, :, 0], c4[:, :, 1])  # LL
                nc.vector.tensor_add(wv[:, 1], d4[:, :, 0], d4[:, :, 1])  # LH
                nc.vector.tensor_sub(wv[:, 2], c4[:, :, 0], c4[:, :, 1])  # HL
                nc.vector.tensor_sub(wv[:, 3], d4[:, :, 0], d4[:, :, 1])  # HH

                # one merged 2MB store (4KB runs per partition+band)
                ov = o[:, i0 : i0 + CI].rearrange(
                    "b (s c) (g j) q -> s (c g) b (j q)", c=CI, j=8
                )[0]
                nc.scalar.dma_start(
                    out=ov, in_=ws[:].rearrange("p (b jq) -> p b jq", b=4)
                )

            def emit_pe_a(ip0):
                """2 pe images ip0, ip0+1: loads, PE vertical matmuls, ACT
                evac. Returns the evac tile for the deferred DVE/store."""
                xv = xp[ip0 : ip0 + 2].rearrange(
                    "(s i) (c p) w -> s c p i w", i=2, p=128
                )[0]
                es = pev_pool.tile([128, 4096], f16, tag="es")
                for c in range(4):
                    xt = px_pool.tile([128, 1024], f16, tag="px")
                    nc.sync.dma_start(
                        out=xt[:].rearrange("p (i w) -> p i w", i=2), in_=xv[c]
                    )
                    pt = ps_pool.tile([128, 1024], f32, tag="ps")
                    for i in range(2):  # image -> psum partition half
                        for par in range(2):  # band-row parity -> free half
                            nc.tensor.matmul(
                                pt[i * 64 : (i + 1) * 64,
                                   par * 512 : (par + 1) * 512],
                                wt[:, par * 64 : (par + 1) * 64],
                                xt[:, i * 512 : (i + 1) * 512],
                                start=True,
                                stop=True,
                            )
                    nc.scalar.copy(es[:, c * 1024 : (c + 1) * 1024], pt[:])
                return es

            def emit_pe_b(ip0, es):
                """Deferred DVE horizontal + stores for pe images ip0, ip0+1.
                Each 512-elem block of es is (eo, 256) thanks to the host's
                per-row column deinterleave -> unit-stride DVE 2x ops."""
                sa = pout_pool.tile([128, 2048], f16, tag="sa")
                sd = pout_pool.tile([128, 2048], f16, tag="sd")
                e3 = es[:].rearrange("p (k eo w) -> p k eo w", eo=2, w=256)
                nc.vector.tensor_add(sa[:], e3[:, :, 0], e3[:, :, 1])
                nc.vector.tensor_sub(sd[:], e3[:, :, 0], e3[:, :, 1])

                sav = sa[:].rearrange("p (c rq) -> p c rq", c=4)
                sdv = sd[:].rearrange("p (c rq) -> p c rq", c=4)
                for i in range(2):
                    ob = {
                        b: o[bi, N_FAT + ip0 + i].rearrange(
                            "(c p r) q -> p c (r q)", p=32, r=2
                        )
                        for b, bi in BAND_IDX.items()
                    }
                    lo, hi = 64 * i, 64 * i + 32
                    nc.scalar.dma_start(out=ob["ll"], in_=sav[lo : lo + 32])
                    nc.sync.dma_start(out=ob["lh"], in_=sdv[lo : lo + 32])
                    nc.sync.dma_start(out=ob["hl"], in_=sav[hi : hi + 32])
                    nc.scalar.dma_start(out=ob["hh"], in_=sdv[hi : hi + 32])

            # 11 fat units + 10 pe units, interleaved; each unit's ACT work
            # (upcast / evac) is emitted before the previous unit's store.
            pend_fat = None  # (i0, xt)
            pend_pe = None   # (ip0, es)
            fat_i0 = 0
            pe_i0 = 0
            for k in range(11):
                xt = emit_fat_load(fat_i0)
                if pend_fat is not None:
                    emit_fat_main(*pend_fat)
                pend_fat = (fat_i0, xt)
                fat_i0 += CI
                if k < 10:
                    es = emit_pe_a(pe_i0)
                    if pend_pe is not None:
                        emit_pe_b(*pend_pe)
                    pend_pe = (pe_i0, es)
                    pe_i0 += 2
            emit_fat_main(*pend_fat)
            emit_pe_b(*pend_pe)
            assert fat_i0 == N_FAT and pe_i0 == N_PE

    nc.compile()
    return nc


_NC_CACHE = {}


def _get_nc(n_images=64):
    if n_images not in _NC_CACHE:
        _NC_CACHE[n_images] = build_nc(n_images)
    return _NC_CACHE[n_images]


def prep_in_maps(x):
    """Host-side input prep: fp16 cast with the Haar /2 folded in (exact),
    even/odd column deinterleave, and fp8 quantization of the R8-row slice
    of the fat images (all pure permutation + dtype casts)."""
    import ml_dtypes

    x = np.asarray(x)
    assert x.shape == (N_CORES, 64, IMG, IMG), x.shape
    xh = (x * np.float32(0.5)).astype(np.float16)
    # fat images: [core, img, g, u, w', eo] -> even/odd planes
    x6 = xh[:, :N_FAT].reshape(N_CORES, N_FAT, 32, 16, 256, 2)
    even = x6[..., 0]
    odd = x6[..., 1]
    x8 = np.ascontiguousarray(even[:, :, :, :R8, :]).astype(
        ml_dtypes.float8_e4m3
    ).reshape(N_CORES, N_FAT, 32, NF8)
    xf = np.ascontiguousarray(np.concatenate(
        [
            even[:, :, :, R8:, :].reshape(N_CORES, N_FAT, 32, -1),
            odd.reshape(N_CORES, N_FAT, 32, -1),
        ],
        axis=-1,
    ))
    assert xf.shape[-1] == NFF, xf.shape
    # pe images: per-row column deinterleave [row] -> [even 256 | odd 256]
    xpe = np.ascontiguousarray(
        xh[:, N_FAT:].reshape(N_CORES, N_PE, IMG, 256, 2).transpose(
            0, 1, 2, 4, 3
        )
    ).reshape(N_CORES, N_PE, IMG, IMG)
    w = make_w()
    return [
        {"x8": x8[i], "xf": xf[i], "xp": xpe[i], "wm": w}
        for i in range(N_CORES)
    ]


def kernel(x, **_unused_matrices):
    """Full-input entry point: x [8, 64, 512, 512] f32 -> (LL, LH, HL, HH)."""
    _ensure_concourse()
    from concourse.bass_utils import run_bass_kernel_spmd

    in_maps = prep_in_maps(x)
    nc = _get_nc(64)
    try:
        res = run_bass_kernel_spmd(nc, in_maps, core_ids=list(range(N_CORES)))
    except ImportError:
        # trace=True was forced via BASS_TRACE but this environment lacks the
        # NTFF profiling hook; run untraced instead of failing.
        import os

        os.environ["BASS_NEVER_TRACE"] = "1"
        res = run_bass_kernel_spmd(nc, in_maps, core_ids=list(range(N_CORES)))
    r = res.results
    return tuple(
        np.stack([r[i]["o"][BAND_IDX[b]] for i in range(N_CORES)]).astype(
            np.float32
        )
        for b in BANDS
    )


# revision 21
# speedup vs baseline: 1.0408x; 1.0408x over previous
"""Haar DWT (512x512, levels=1) on 8 Trainium2 NeuronCores.

Input  x: [8, 64, 512, 512] f32  (plus the four Haar band matrices, which
are fixed/deterministic and therefore folded into the kernel math).
Output: (LL, LH, HL, HH), each [8, 64, 256, 256] f32.

Strategy: pure data parallel over the batch dim (core i handles x[i]).
The Haar /2 is folded into the host-side cast (x*0.5, exact).

Two cooperating pipelines (per core), both fed by host-side column
deinterleaving (a pure permutation folded into the cast/copy pass):

* fat path (44 images, units of 4): even/odd column halves stored
  contiguously, so the horizontal butterfly is `even +- odd` on
  unit-stride fp16 (DVE 2x mode) and the vertical butterfly pairs
  adjacent rows within a partition (also 2x). 6 DVE ops per unit,
  ~9.3us. Rows 0..7 of every 16-row group ship their even-column half
  as fp8-e4m3, upcast to fp16 on ACT before the butterflies: exactly 2
  of 4 butterfly inputs fp8 for half the outputs -> rel_err ~1.33e-2
  (sim+HW confirmed), evenly spread over the bands, under the 2e-2
  gate. Input traffic 32MB -> 29.25MB/core.
* pe path (20 images, units of 2): rows-on-partitions load, PE vertical
  butterfly as a [128,128] +-1 band matmul (two band rows per psum
  partition), ACT evacuates PSUM->fp16, DVE does ONLY the horizontal
  `even +- odd` (2x mode thanks to per-row column deinterleave) --
  1.25us/img of DVE instead of 2.3.

DVE busy ~129us, ACT ~85us, PE ~50us, DMA ~61MB at the measured
~420GB/s/core cap -> ~150us; the mix keeps every engine just under the
DMA roofline.

Hard-won scheduling facts (measured): DVE ops need <=2 free AP dims or
they drop out of 2x mode; GpSimd must stay COMPLETELY idle (any op or
DMA trigger there costs ~+30us -- the Q7 cores back the DMA descriptor
path); fio bufs=3 is load-bearing (bufs=2 serializes, +30us); 4KB DMA
runs beat 8KB; one merged store dma_start per unit on the scalar queue
beats fine-grained or multi-queue stores; each unit's ACT work (upcast,
evac) is emitted one unit ahead of the previous unit's store trigger to
avoid ACT head-of-line blocking.
"""

import numpy as np


def _ensure_concourse():
    try:
        import concourse.bass  # noqa: F401
    except ImportError:
        import sys

        for p in ("/opt/trn_rl_repo", "/root/.axon_site/_ro/trn_rl_repo"):
            if p not in sys.path:
                sys.path.append(p)
        import concourse.bass  # noqa: F401


N_CORES = 8
IMG = 512  # image height == width
BANDS = ("ll", "lh", "hl", "hh")
# band order inside the merged fat-path output tensor
BAND_IDX = {"ll": 0, "lh": 1, "hl": 2, "hh": 3}

N_FAT = 44      # images through the fat path (units of 4)
N_PE = 20       # images through the pe path (units of 2)
R8 = 8          # rows per 16-row group whose even-col half ships as fp8
NF8 = R8 * 256  # fp8 elems per partition (upcast target xt[:, :NF8])
NFF = 8192 - NF8  # fp16 elems per partition loaded directly


def make_w():
    """[128,128] fp16 weights [W_e | W_o] for the PE vertical stage.

    W_e col m<32 sums input row pair (4m, 4m+1) -> EVEN band row 2m of L;
    col 32+m difs the same pair -> even band row of H. W_o handles the odd
    band rows (pairs (4m+2, 4m+3)). Using both per psum tile puts TWO
    consecutive band rows on each psum partition (free halves)."""
    w = np.zeros((128, 128), dtype=np.float16)
    for m in range(32):
        for par, base in ((0, 0), (1, 64)):
            r0 = 4 * m + 2 * par
            w[r0, base + m] = 1.0
            w[r0 + 1, base + m] = 1.0
            w[r0, base + 32 + m] = 1.0
            w[r0 + 1, base + 32 + m] = -1.0
    return w


def build_nc(n_images=64):
    """Build the single-core Bass program (SPMD: same program on all cores)."""
    _ensure_concourse()
    from concourse import bacc, mybir
    from concourse.tile import TileContext

    f16 = mybir.dt.float16
    f32 = mybir.dt.float32
    f8 = mybir.dt.float8e4
    # NOTE: keep enable_partition_id at its default (True). Building with
    # False removes a ~3.7 us preamble TENSOR_LOAD but the axon PJRT execute
    # path requires the trailing partition-id parameter and the NEFF faults
    # with NRT_EXEC_UNIT_UNRECOVERABLE without it.
    nc = bacc.Bacc("TRN2", target_bir_lowering=False, debug=False)

    # fat-path inputs, per 4-image unit partition (c g) of 128:
    #   x8: rows 0..R8-1 even cols, fp8       -> NF8 B contiguous/partition
    #   xf: rows R8..15 even cols ++ all odd cols, fp16 -> 2*NFF B contiguous
    x8 = nc.dram_tensor("x8", [N_FAT, 32, NF8], f8, kind="ExternalInput")
    xf = nc.dram_tensor("xf", [N_FAT, 32, NFF], f16, kind="ExternalInput")
    # pe-path input: row-major, each row stored [even cols | odd cols]
    xp = nc.dram_tensor("xp", [N_PE, IMG, IMG], f16, kind="ExternalInput")
    wm = nc.dram_tensor("wm", [128, 128], f16, kind="ExternalInput")
    o = nc.dram_tensor("o", [4, n_images, IMG // 2, IMG // 2], f16,
                       kind="ExternalOutput")

    CI = 4
    FX = 2048 * CI  # free elems per partition of the assembled input tile

    with TileContext(nc) as tc:
        with (
            tc.tile_pool(name="const", bufs=1) as const_pool,
            tc.tile_pool(name="fio", bufs=3) as fio_pool,
            tc.tile_pool(name="f8io", bufs=3) as f8_pool,
            tc.tile_pool(name="fmid", bufs=3) as fmid_pool,
            tc.tile_pool(name="fws", bufs=3) as fws_pool,
            tc.tile_pool(name="pxin", bufs=4) as px_pool,
            tc.tile_pool(name="pev", bufs=2) as pev_pool,
            tc.tile_pool(name="pout", bufs=2) as pout_pool,
            tc.tile_pool(name="ps", bufs=4, space="PSUM") as ps_pool,
        ):
            wt = const_pool.tile([128, 128], f16, tag="w")
            nc.sync.dma_start(out=wt[:], in_=wm[:])

            def emit_fat_load(i0):
                """Fat-path load + ACT upcast (one unit ahead of its
                compute/store block: ACT head-of-line blocking otherwise)."""
                xt = fio_pool.tile([128, FX], f16, tag="x")
                x8t = f8_pool.tile([128, NF8], f8, tag="x8")
                xv8 = x8[i0 : i0 + CI].rearrange("c g m -> (c g) m")
                nc.sync.dma_start(out=x8t[:], in_=xv8)
                nc.scalar.copy(out=xt[:, :NF8], in_=x8t[:])
                xvf = xf[i0 : i0 + CI].rearrange("c g m -> (c g) m")
                for k in range(NFF // 2048):
                    nc.sync.dma_start(
                        out=xt[:, NF8 + k * 2048 : NF8 + (k + 1) * 2048],
                        in_=xvf[:, k * 2048 : (k + 1) * 2048],
                    )
                return xt

            def emit_fat_main(i0, xt):
                # horizontal butterfly: even half +- odd half (DVE 2x mode)
                xtv = xt[:].rearrange("p (eo m) -> p eo m", eo=2)
                cs = fmid_pool.tile([128, FX // 2], f16, tag="cs")
                cd = fmid_pool.tile([128, FX // 2], f16, tag="cd")
                nc.vector.tensor_add(cs[:], xtv[:, 0], xtv[:, 1])
                nc.vector.tensor_sub(cd[:], xtv[:, 0], xtv[:, 1])

                # vertical butterfly: adjacent row pairs within a partition
                ws = fws_pool.tile([128, FX], f16, tag="ws")
                wv = ws[:].rearrange("p (b j w) -> p b j w", b=4, w=256)
                c4 = cs[:].rearrange("p (j eo w) -> p j eo w", eo=2, w=256)
                d4 = cd[:].rearrange("p (j eo w) -> p j eo w", eo=2, w=256)
                nc.vector.tensor_add(wv[:, 0], c4[:, :, 0], c4[:, :, 1])  # LL
                nc.vector.tensor_add(wv[:, 1], d4[:, :, 0], d4[:, :, 1])  # LH
                nc.vector.tensor_sub(wv[:, 2], c4[:, :, 0], c4[:, :, 1])  # HL
                nc.vector.tensor_sub(wv[:, 3], d4[:, :, 0], d4[:, :, 1])  # HH

                # one merged 2MB store (4KB runs per partition+band)
                ov = o[:, i0 : i0 + CI].rearrange(
                    "b (s c) (g j) q -> s (c g) b (j q)", c=CI, j=8
                )[0]
                nc.scalar.dma_start(
                    out=ov, in_=ws[:].rearrange("p (b jq) -> p b jq", b=4)
                )

            def emit_pe_a(ip0):
                """2 pe images ip0, ip0+1: loads, PE vertical matmuls, ACT
                evac. Returns the evac tile for the deferred DVE/store."""
                xv = xp[ip0 : ip0 + 2].rearrange(
                    "(s i) (c p) w -> s c p i w", i=2, p=128
                )[0]
                es = pev_pool.tile([128, 4096], f16, tag="es")
                for c in range(4):
                    xt = px_pool.tile([128, 1024], f16, tag="px")
                    nc.sync.dma_start(
                        out=xt[:].rearrange("p (i w) -> p i w", i=2), in_=xv[c]
                    )
                    pt = ps_pool.tile([128, 1024], f32, tag="ps")
                    for i in range(2):  # image -> psum partition half
                        for par in range(2):  # band-row parity -> free half
                            nc.tensor.matmul(
                                pt[i * 64 : (i + 1) * 64,
                                   par * 512 : (par + 1) * 512],
                                wt[:, par * 64 : (par + 1) * 64],
                                xt[:, i * 512 : (i + 1) * 512],
                                start=True,
                                stop=True,
                            )
                    nc.scalar.copy(es[:, c * 1024 : (c + 1) * 1024], pt[:])
                return es

            def emit_pe_b(ip0, es):
                """Deferred DVE horizontal + stores for pe images ip0, ip0+1.
                Each 512-elem block of es is (eo, 256) thanks to the host's
                per-row column deinterleave -> unit-stride DVE 2x ops."""
                sa = pout_pool.tile([128, 2048], f16, tag="sa")
                sd = pout_pool.tile([128, 2048], f16, tag="sd")
                e3 = es[:].rearrange("p (k eo w) -> p k eo w", eo=2, w=256)
                nc.vector.tensor_add(sa[:], e3[:, :, 0], e3[:, :, 1])
                nc.vector.tensor_sub(sd[:], e3[:, :, 0], e3[:, :, 1])

                sav = sa[:].rearrange("p (c rq) -> p c rq", c=4)
                sdv = sd[:].rearrange("p (c rq) -> p c rq", c=4)
                for i in range(2):
                    ob = {
                        b: o[bi, N_FAT + ip0 + i].rearrange(
                            "(c p r) q -> p c (r q)", p=32, r=2
                        )
                        for b, bi in BAND_IDX.items()
                    }
                    # all on scalar: a store trigger on the sync queue waits
                    # on DVE and head-of-line-blocks every later load
                    lo, hi = 64 * i, 64 * i + 32
                    nc.scalar.dma_start(out=ob["ll"], in_=sav[lo : lo + 32])
                    nc.scalar.dma_start(out=ob["lh"], in_=sdv[lo : lo + 32])
                    nc.scalar.dma_start(out=ob["hl"], in_=sav[hi : hi + 32])
                    nc.scalar.dma_start(out=ob["hh"], in_=sdv[hi : hi + 32])

            # 11 fat units + 10 pe units, interleaved; each unit's ACT work
            # (upcast / evac) is emitted before the previous unit's store.
            pend_fat = None  # (i0, xt)
            pend_pe = None   # (ip0, es)
            fat_i0 = 0
            pe_i0 = 0
            for k in range(11):
                xt = emit_fat_load(fat_i0)
                if pend_fat is not None:
                    emit_fat_main(*pend_fat)
                pend_fat = (fat_i0, xt)
                fat_i0 += CI
                if k < 10:
                    es = emit_pe_a(pe_i0)
                    if pend_pe is not None:
                        emit_pe_b(*pend_pe)
                    pend_pe = (pe_i0, es)
                    pe_i0 += 2
            emit_fat_main(*pend_fat)
            emit_pe_b(*pend_pe)
            assert fat_i0 == N_FAT and pe_i0 == N_PE

    nc.compile()
    return nc


_NC_CACHE = {}


def _get_nc(n_images=64):
    if n_images not in _NC_CACHE:
        _NC_CACHE[n_images] = build_nc(n_images)
    return _NC_CACHE[n_images]


def prep_in_maps(x):
    """Host-side input prep: fp16 cast with the Haar /2 folded in (exact),
    even/odd column deinterleave, and fp8 quantization of the R8-row slice
    of the fat images (all pure permutation + dtype casts)."""
    import ml_dtypes

    x = np.asarray(x)
    assert x.shape == (N_CORES, 64, IMG, IMG), x.shape
    xh = (x * np.float32(0.5)).astype(np.float16)
    # fat images: [core, img, g, u, w', eo] -> even/odd planes
    x6 = xh[:, :N_FAT].reshape(N_CORES, N_FAT, 32, 16, 256, 2)
    even = x6[..., 0]
    odd = x6[..., 1]
    x8 = np.ascontiguousarray(even[:, :, :, :R8, :]).astype(
        ml_dtypes.float8_e4m3
    ).reshape(N_CORES, N_FAT, 32, NF8)
    xf = np.ascontiguousarray(np.concatenate(
        [
            even[:, :, :, R8:, :].reshape(N_CORES, N_FAT, 32, -1),
            odd.reshape(N_CORES, N_FAT, 32, -1),
        ],
        axis=-1,
    ))
    assert xf.shape[-1] == NFF, xf.shape
    # pe images: per-row column deinterleave [row] -> [even 256 | odd 256]
    xpe = np.ascontiguousarray(
        xh[:, N_FAT:].reshape(N_CORES, N_PE, IMG, 256, 2).transpose(
            0, 1, 2, 4, 3
        )
    ).reshape(N_CORES, N_PE, IMG, IMG)
    w = make_w()
    return [
        {"x8": x8[i], "xf": xf[i], "xp": xpe[i], "wm": w}
        for i in range(N_CORES)
    ]


def kernel(x, **_unused_matrices):
    """Full-input entry point: x [8, 64, 512, 512] f32 -> (LL, LH, HL, HH)."""
    _ensure_concourse()
    from concourse.bass_utils import run_bass_kernel_spmd

    in_maps = prep_in_maps(x)
    nc = _get_nc(64)
    try:
        res = run_bass_kernel_spmd(nc, in_maps, core_ids=list(range(N_CORES)))
    except ImportError:
        # trace=True was forced via BASS_TRACE but this environment lacks the
        # NTFF profiling hook; run untraced instead of failing.
        import os

        os.environ["BASS_NEVER_TRACE"] = "1"
        res = run_bass_kernel_spmd(nc, in_maps, core_ids=list(range(N_CORES)))
    r = res.results
    return tuple(
        np.stack([r[i]["o"][BAND_IDX[b]] for i in range(N_CORES)]).astype(
            np.float32
        )
        for b in BANDS
    )


# revision 22
# speedup vs baseline: 1.1581x; 1.1127x over previous
"""Haar DWT (512x512, levels=1) on 8 Trainium2 NeuronCores.

Input  x: [8, 64, 512, 512] f32  (plus the four Haar band matrices, which
are fixed/deterministic and therefore folded into the kernel math).
Output: (LL, LH, HL, HH), each [8, 64, 256, 256] f32.

Strategy: pure data parallel over the batch dim (core i handles x[i]).
All HBM traffic is fp16 (grading tolerance is 2e-2 rel; fp16 adds ~4e-4)
and the Haar /2 is folded into the host-side cast (x*0.5, exact).

The key layout trick: the host pre-deinterleaves even/odd image COLUMNS
(a pure permutation, folded into the same host-side cast/copy pass that
already exists for the fp16 conversion). With the two column phases
stored as separate contiguous halves, the horizontal butterfly becomes
`even_half +- odd_half` on unit-stride fp16 operands, and the vertical
butterfly pairs adjacent rows within a partition (gappy but unit-stride
inner dim). All six DVE ops per tile therefore run in the 2x perf mode
(2-byte dtype + innermost stride 1 + <=2 free AP dims), unlike the naive
in-order layout whose stride-2 horizontal pass is stuck at 1x. DVE busy
~146us, under the ~152us DMA roofline (64MB/core at the measured
~420GB/s/core aggregate cap), so no PE/ACT assist is needed.

DMA: per unit of 4 images, loads are 4x 512KB dma_starts with 4KB
descriptor runs on the sync queue (4KB is the measured packet sweet
spot; 8KB runs and multi-queue/fine-grained stores all measured slower
end-to-end) and the store is one merged 2MB dma_start on the scalar
queue (bands in one dram tensor, 4KB runs).

Hard-won scheduling facts (each measured as ~+30us when violated):
 - GpSimd must stay COMPLETELY idle -- any op or DMA trigger there
   starves the DMA descriptor path (Q7 cores back it).
 - fio bufs=3 is load-bearing; bufs=2 serializes the pipeline.
 - DVE ops with 3+ free AP dims drop out of 2x mode (1.5ns/elem).
Variants tried and measurably worse: fp8-e4m3 half-input with ACT
upcast (DVE becomes the pacer, +7us), PE-matmul vertical stage for a
subset of images (baseline-style hybrid, +24us), 8KB runs (+16us),
band-interleaved output layout with per-band store chunks (+30us).
"""

import numpy as np


def _ensure_concourse():
    try:
        import concourse.bass  # noqa: F401
    except ImportError:
        import sys

        for p in ("/opt/trn_rl_repo", "/root/.axon_site/_ro/trn_rl_repo"):
            if p not in sys.path:
                sys.path.append(p)
        import concourse.bass  # noqa: F401


N_CORES = 8
IMG = 512  # image height == width
BANDS = ("ll", "lh", "hl", "hh")
# band order inside the merged output tensor
BAND_IDX = {"ll": 0, "lh": 1, "hl": 2, "hh": 3}


def build_nc(n_images=64):
    """Build the single-core Bass program (SPMD: same program on all cores)."""
    _ensure_concourse()
    from concourse import bacc, mybir
    from concourse.tile import TileContext

    f16 = mybir.dt.float16
    # NOTE: keep enable_partition_id at its default (True). Building with
    # False removes a ~3.7 us preamble TENSOR_LOAD but the axon PJRT execute
    # path requires the trailing partition-id parameter and the NEFF faults
    # with NRT_EXEC_UNIT_UNRECOVERABLE without it.
    nc = bacc.Bacc("TRN2", target_bir_lowering=False, debug=False)

    # x layout (host-prepped): [img, g=32, eo=2, u=16, w=256] so that each
    # of the 128 partitions (c g) of a 4-image unit owns 16KB contiguous
    # DRAM: 16 consecutive rows' even-column half then odd-column half.
    x = nc.dram_tensor("x", [n_images, 32, 2, 16, 256], f16,
                       kind="ExternalInput")
    o = nc.dram_tensor("o", [4, n_images, IMG // 2, IMG // 2], f16,
                       kind="ExternalOutput")

    CI = 4          # images per unit
    FX = 2048 * CI  # free elems per partition of the input tile

    with TileContext(nc) as tc:
        with (
            tc.tile_pool(name="fio", bufs=3) as fio_pool,
            tc.tile_pool(name="fmid", bufs=3) as fmid_pool,
            tc.tile_pool(name="fws", bufs=3) as fws_pool,
        ):
            def emit_unit(i0):
                xv = x[i0 : i0 + CI].rearrange("c g eo u w -> (c g) (eo u w)")
                xt = fio_pool.tile([128, FX], f16, tag="x")
                # 4KB descriptor runs (measured best per-packet rate; 16KB
                # packets degrade ~20% under load, 2KB measured 20.5 B/ns
                # vs 4KB's 23-25)
                for k in range(FX // 2048):
                    nc.sync.dma_start(
                        out=xt[:, k * 2048 : (k + 1) * 2048],
                        in_=xv[:, k * 2048 : (k + 1) * 2048],
                    )

                # horizontal butterfly: even half +- odd half, all unit
                # stride -> 2x mode. cs = col sums, cd = col difs.
                xtv = xt[:].rearrange("p (eo m) -> p eo m", eo=2)
                cs = fmid_pool.tile([128, FX // 2], f16, tag="cs")
                cd = fmid_pool.tile([128, FX // 2], f16, tag="cd")
                nc.vector.tensor_add(cs[:], xtv[:, 0], xtv[:, 1])
                nc.vector.tensor_sub(cd[:], xtv[:, 0], xtv[:, 1])

                # vertical butterfly: adjacent row pairs within a partition
                # (inner dim w=256 unit stride -> still 2x mode), written
                # into the four band blocks of one merged store tile.
                ws = fws_pool.tile([128, FX], f16, tag="ws")
                wv = ws[:].rearrange("p (b j w) -> p b j w", b=4, w=256)
                c4 = cs[:].rearrange("p (j eo w) -> p j eo w", eo=2, w=256)
                d4 = cd[:].rearrange("p (j eo w) -> p j eo w", eo=2, w=256)
                nc.vector.tensor_add(wv[:, 0], c4[:, :, 0], c4[:, :, 1])  # LL
                nc.vector.tensor_add(wv[:, 1], d4[:, :, 0], d4[:, :, 1])  # LH
                nc.vector.tensor_sub(wv[:, 2], c4[:, :, 0], c4[:, :, 1])  # HL
                nc.vector.tensor_sub(wv[:, 3], d4[:, :, 0], d4[:, :, 1])  # HH

                # merged 2MB store, 4KB runs per (partition, band)
                ov = o[:, i0 : i0 + CI].rearrange(
                    "b (s c) (g j) q -> s (c g) b (j q)", c=CI, j=8
                )[0]
                nc.scalar.dma_start(
                    out=ov, in_=ws[:].rearrange("p (b jq) -> p b jq", b=4)
                )

            for i0 in range(0, n_images, CI):
                emit_unit(i0)

    nc.compile()
    return nc


_NC_CACHE = {}


def _get_nc(n_images=64):
    if n_images not in _NC_CACHE:
        _NC_CACHE[n_images] = build_nc(n_images)
    return _NC_CACHE[n_images]


def prep_in_maps(x):
    """Host-side input prep: fp16 cast with the Haar /2 folded in (exact),
    plus the even/odd column deinterleave (pure permutation)."""
    x = np.asarray(x)
    assert x.shape == (N_CORES, 64, IMG, IMG), x.shape
    xh = (x * np.float32(0.5)).astype(np.float16)
    # [core, img, g, u, w', eo] -> [core, img, g, eo, u, w']
    xp = np.ascontiguousarray(
        xh.reshape(N_CORES, 64, 32, 16, 256, 2).transpose(0, 1, 2, 5, 3, 4)
    )
    return [{"x": xp[i]} for i in range(N_CORES)]


def kernel(x, **_unused_matrices):
    """Full-input entry point: x [8, 64, 512, 512] f32 -> (LL, LH, HL, HH)."""
    _ensure_concourse()
    from concourse.bass_utils import run_bass_kernel_spmd

    in_maps = prep_in_maps(x)
    nc = _get_nc(64)
    try:
        res = run_bass_kernel_spmd(nc, in_maps, core_ids=list(range(N_CORES)))
    except ImportError:
        # trace=True was forced via BASS_TRACE but this environment lacks the
        # NTFF profiling hook; run untraced instead of failing.
        import os

        os.environ["BASS_NEVER_TRACE"] = "1"
        res = run_bass_kernel_spmd(nc, in_maps, core_ids=list(range(N_CORES)))
    r = res.results
    return tuple(
        np.stack([r[i]["o"][BAND_IDX[b]] for i in range(N_CORES)]).astype(
            np.float32
        )
        for b in BANDS
    )
